# revision 1
# baseline (speedup 1.0000x reference)
"""Trainium2 Bass kernel for 2-layer GIN (DGI) message passing.

Reference computation (per layer, mean aggregation, eps=0):
    agg = segment_sum(h[src], dst) / max(deg,1)
    h'  = relu((h + agg) @ W.T + b)

Linearity trick: (h + agg(h)) @ W.T = y + agg(y) with y = h @ W.T, so both
layers aggregate 128-wide features:
    y1 = h @ W1.T          ; h1 = relu(y1 + agg(y1)*inv_deg + b1)
    y2 = h1 @ W2.T         ; h2 = relu(y2 + agg(y2)*inv_deg + b2)

Distribution: nodes sharded 12500/core across 8 cores (degree-balanced
permutation, (range, src-group) cells packed by a vector LPT greedy),
per-layer AllGather of the projected features y (the gather table),
per-edge messages fetched with GPSIMD dma_gather (int16 indices -> 4
source windows), segment-sum via one-hot selection-matrix matmuls (fp8)
accumulated in PSUM (edges sorted by destination range). Biases are
folded into the y_own stores so the per-range epilogue is one
scalar_tensor_tensor + relu.

build_program(meta, reps=R) unrolls the whole computation R times; test.py
measures true per-execution device time as the marginal-time slope between
reps=R and reps=1 (a single axon dispatch carries a ~4ms client-side floor
that is not device time).
"""

import math
import time
import numpy as np

import concourse.bass as bass
import concourse.bacc as bacc
import concourse.tile as tile
from concourse import bass2jax, mybir

P = 128
NCORES = 8
N = 100000
E = 3200000
IN_FEATS = 256
H_FEATS = 128
NB = N // NCORES             # 12500 nodes per core
NRANGES = math.ceil(NB / P)  # 98 (last range has 84 slots)
NGROUPS = 4
GROUP_ROWS = N // NGROUPS    # 25000 (< 32768, fits int16 index)

MSG_BF16 = True              # gather table + messages in bf16
W_BATCH = 7                  # ranges per gather instruction (SBUF budget)

# AllGather chunking: the y table is split at range SPLIT_R into two
# contiguous tensors (A = rows [0, SPLIT_I) of every core, B = the rest) so
# each chunk's collective output is contiguous and can be issued as soon as
# its producing ranges finish. Gather windows: group 0/1 = cores 0-3 / 4-7
# of A, group 2/3 = cores 0-3 / 4-7 of B (all < 2^15, int16-safe).
AG_CHUNKED = False           # chunked AG measured slower (small-cc penalty)
SPLIT_R = 49
SPLIT_I = SPLIT_R * P        # 6272
W_A = SPLIT_I                # rows per core in A
W_B = NB - SPLIT_I           # 6228 rows per core in B
ROWS_A = NCORES * W_A
ROWS_B = NCORES * W_B
if AG_CHUNKED:
    GSIZE = [ROWS_A // 2, ROWS_A // 2, ROWS_B // 2, ROWS_B // 2]
    GBASE = [0, ROWS_A // 2, 0, ROWS_B // 2]
else:
    GSIZE = [GROUP_ROWS] * NGROUPS
    GBASE = [g * GROUP_ROWS for g in range(NGROUPS)]


def _bf16():
    import ml_dtypes
    return ml_dtypes.bfloat16


# ---------------------------------------------------------------- host side

def _snake_fill(items, nbins, caps):
    """Distribute items (in given order) over bins in snake order, skipping
    full bins. Returns list of lists."""
    buckets = [[] for _ in range(nbins)]
    caps = list(caps)
    b, d = 0, 1
    for it in items:
        while caps[b] == 0:
            nb = b + d
            if nb < 0 or nb >= nbins:
                d = -d
                nb = b + d
            b = nb
        buckets[b].append(it)
        caps[b] -= 1
        nb = b + d
        if nb < 0 or nb >= nbins:
            d = -d
        else:
            b = nb
    return buckets


def _balance_partition(deg, src, dst):
    """old node id -> new node id; new layout: core*NB + within-core index,
    range r = within-core indices [r*128, min((r+1)*128, NB)).

    Phase 1 snake-fills nodes over cores by in-degree (balances per-core
    edge totals). Phase 2 packs each core's nodes into ranges so that each
    (range, src-group) cell's edge count stays <= 1024 (8 tiles of 128) —
    a vector LPT greedy over the per-node group-degree vectors."""
    order = np.argsort(-deg, kind="stable")
    cores = _snake_fill(order, NCORES, [NB] * NCORES)
    core_of = np.empty(N, np.int64)
    for ci in range(NCORES):
        core_of[np.asarray(cores[ci])] = ci
    g_src = core_of[src] // 2  # src group = core pair (position // GROUP_ROWS)
    dvec = np.bincount(dst * NGROUPS + g_src, minlength=N * NGROUPS)
    dvec = dvec.reshape(N, NGROUPS)

    caps = np.array([min(P, NB - r * P) for r in range(NRANGES)], np.int64)
    offs = np.concatenate([[0], np.cumsum(caps)])[:NRANGES]
    # Hard cap of 8 tiles (1024 edges) per (range, group) cell on the full
    # ranges; the final 84-node range is the designated spill bucket (its
    # natural load is ~0.7x, so a handful of spill tiles land there).
    CAP = np.full((NRANGES, 1), float(P * 8))
    CAP[NRANGES - 1] = np.inf
    perm = np.empty(N, np.int64)
    for ci in range(NCORES):
        nodes = np.asarray(cores[ci])
        nodes = nodes[np.argsort(-deg[nodes], kind="stable")]
        d = dvec[nodes].astype(np.float64)
        loads = np.zeros((NRANGES, NGROUPS), np.float64)
        counts = np.zeros(NRANGES, np.int64)
        pos = np.zeros(NRANGES, np.int64)
        for i in range(len(nodes)):
            cand = loads + d[i]
            over = np.where(np.isinf(CAP), 0.0, np.maximum(cand - CAP, 0.0))
            score = over.sum(1) * 1e9 + cand.max(1)
            score[counts >= caps] = np.inf
            b = int(np.argmin(score))
            loads[b] += d[i]
            counts[b] += 1
            perm[nodes[i]] = ci * NB + offs[b] + pos[b]
            pos[b] += 1
    return perm


def preprocess(h, W1, b1, W2, b2, src, dst):
    src = np.asarray(src).astype(np.int64)
    dst = np.asarray(dst).astype(np.int64)
    deg = np.bincount(dst, minlength=N).astype(np.int64)
    inv_deg = (1.0 / np.maximum(deg, 1.0)).astype(np.float32)

    perm = _balance_partition(deg, src, dst)  # old -> new
    inv_perm = np.empty(N, np.int64)
    inv_perm[perm] = np.arange(N)

    src_n = perm[src]
    dst_n = perm[dst]

    core_e = dst_n // NB
    if AG_CHUNKED:
        c_src = src_n // NB
        i_src = src_n - c_src * NB
        half = (i_src >= SPLIT_I).astype(np.int64)
        grp_e = half * 2 + (c_src >= NCORES // 2)
        # window-relative table row (A: c*W_A + i, B: c*W_B + (i - SPLIT_I),
        # both modulo the 4-core half each group covers)
        rel_e = ((c_src % (NCORES // 2)) * np.where(half == 1, W_B, W_A)
                 + (i_src - half * SPLIT_I))
    else:
        grp_e = src_n // GROUP_ROWS
        rel_e = src_n - grp_e * GROUP_ROWS
    order = np.lexsort((dst_n, grp_e, core_e))
    src_n, dst_n, core_e, grp_e, rel_e = (
        src_n[order], dst_n[order], core_e[order], grp_e[order], rel_e[order])

    loc = dst_n - core_e * NB
    rng_e = loc // P
    cell_id = ((core_e * NRANGES) + rng_e) * NGROUPS + grp_e
    counts = np.bincount(cell_id, minlength=NCORES * NRANGES * NGROUPS)
    counts = counts.reshape(NCORES, NRANGES, NGROUPS)
    T = np.maximum(1, np.ceil(counts.max(axis=0) / P).astype(np.int64))

    CT = T.sum(axis=1)
    CTOFF = np.concatenate([[0], np.cumsum(CT)])
    TOT_G = T.sum(axis=0) * P
    batches = [list(range(b, min(b + W_BATCH, NRANGES)))
               for b in range(0, NRANGES, W_BATCH)]

    in_maps = []
    for c in range(NCORES):
        sel = core_e == c
        s_c, g_c = rel_e[sel], grp_e[sel]
        loc_c = dst_n[sel] - c * NB
        r_c = loc_c // P
        slot_c = loc_c - r_c * P

        idx_g = [np.zeros(TOT_G[g], np.int16) for g in range(NGROUPS)]
        dstp = np.full((P, int(CT.sum())), 255, np.float32)
        for g in range(NGROUPS):
            gsel = g_c == g
            sg, rg, slg = s_c[gsel], r_c[gsel], slot_c[gsel]
            rng_counts = np.bincount(rg, minlength=NRANGES)
            off = 0
            pos = 0
            for r in range(NRANGES):
                cnt = int(rng_counts[r])
                L = int(T[r, g]) * P
                assert cnt <= L, (c, r, g, cnt, L)
                idx_g[g][pos:pos + cnt] = sg[off:off + cnt].astype(np.int16)
                colbase = int(CTOFF[r]) + int(T[r, :g].sum())
                flat = np.full(L, 255, np.float32)
                flat[:cnt] = slg[off:off + cnt]
                dstp[:, colbase:colbase + int(T[r, g])] = (
                    flat.reshape(int(T[r, g]), P).T)
                off += cnt
                pos += L
            assert pos == TOT_G[g]

        idx_wrapped = []
        for g in range(NGROUPS):
            wr = idx_g[g].reshape(-1, 16).T
            idx_wrapped.append(np.tile(wr, (8, 1)).copy())

        # one-hot selection matrices, fp8 (exact 0/1), [P, CTsum*128]
        import ml_dtypes
        S_all = (dstp[:, :, None] == np.arange(P, dtype=np.float32)[None, None, :]
                 ).astype(ml_dtypes.float8_e4m3fn).reshape(P, int(CT.sum()) * P)

        own_old = inv_perm[np.arange(c * NB, (c + 1) * NB)]
        hT = np.ascontiguousarray(h[own_old].T).astype(np.float32)
        ivd = np.ones(NRANGES * P, np.float32)
        ivd[:NB] = inv_deg[own_old]
        invdegT = np.ascontiguousarray(ivd.reshape(NRANGES, P).T)

        m = {
            "hT": hT,
            "invdegT": invdegT,
            "S_all": S_all,
            "W1T": np.ascontiguousarray(W1.T).astype(np.float32),
            "W2T": np.ascontiguousarray(W2.T).astype(np.float32),
            "b1_rep": np.broadcast_to(b1, (P, H_FEATS)).copy().astype(np.float32),
            "b2_rep": np.broadcast_to(b2, (P, H_FEATS)).copy().astype(np.float32),
            "identity": np.eye(P, dtype=np.float32),
        }
        for g in range(NGROUPS):
            m[f"idx{g}"] = idx_wrapped[g]
        in_maps.append(m)

    meta = dict(T=T, CT=CT, CTOFF=CTOFF, TOT_G=TOT_G, batches=batches,
                perm=perm, inv_perm=inv_perm)
    return in_maps, meta


# ------------------------------------------------------------- device build

def build_program(meta, no_collectives=False, skip=(), reps=1):
    T, CT, CTOFF, batches = meta["T"], meta["CT"], meta["CTOFF"], meta["batches"]
    TOT_G = meta["TOT_G"]

    nc = bacc.Bacc("TRN2", target_bir_lowering=False, debug=False,
                   num_devices=NCORES, num_swdge_queues=4)
    f32 = mybir.dt.float32
    i16 = mybir.dt.int16
    mdt = mybir.dt.bfloat16 if MSG_BF16 else f32

    hT_d = nc.dram_tensor("hT", [IN_FEATS, NB], f32, kind="ExternalInput")
    invdegT_d = nc.dram_tensor("invdegT", [P, NRANGES], f32, kind="ExternalInput")
    S_d = nc.dram_tensor("S_all", [P, int(CT.sum()) * P], mybir.dt.float8e4,
                         kind="ExternalInput")
    W1T_d = nc.dram_tensor("W1T", [IN_FEATS, H_FEATS], f32, kind="ExternalInput")
    W2T_d = nc.dram_tensor("W2T", [H_FEATS, H_FEATS], f32, kind="ExternalInput")
    b1_d = nc.dram_tensor("b1_rep", [P, H_FEATS], f32, kind="ExternalInput")
    b2_d = nc.dram_tensor("b2_rep", [P, H_FEATS], f32, kind="ExternalInput")
    ident_d = nc.dram_tensor("identity", [P, P], f32, kind="ExternalInput")
    idx_d = [nc.dram_tensor(f"idx{g}", [P, int(TOT_G[g]) // 16], i16,
                            kind="ExternalInput") for g in range(NGROUPS)]
    out_d = nc.dram_tensor("out", [NB, H_FEATS], f32, kind="ExternalOutput")

    with tile.TileContext(nc) as tc:
        with (
            tc.tile_pool(name="const", bufs=1) as cpool,
            tc.tile_pool(name="sb", bufs=2) as sb,
            tc.tile_pool(name="slab", bufs=2) as slabp,
            tc.tile_pool(name="ps_agg", bufs=2, space="PSUM") as ps_agg,
            tc.tile_pool(name="ps_tr", bufs=2, space="PSUM") as ps_tr,
            tc.tile_pool(name="ps_y2", bufs=2, space="PSUM") as ps_y2,
            tc.tile_pool(name="ps_a", bufs=2, space="PSUM") as ps_a,
            tc.tile_pool(name="dram", bufs=2, space="DRAM") as dram,
        ):

            invdegT_t = cpool.tile([P, NRANGES], f32, tag="ivd")
            nc.sync.dma_start(out=invdegT_t[:], in_=invdegT_d[:])
            W1T_t = [cpool.tile([P, H_FEATS], f32, tag=f"w1_{ch}", name=f"w1_{ch}")
                     for ch in range(IN_FEATS // P)]
            for ch in range(IN_FEATS // P):
                nc.sync.dma_start(out=W1T_t[ch][:],
                                  in_=W1T_d[ch * P:(ch + 1) * P, :])
            W2T_t = cpool.tile([P, H_FEATS], f32, tag="w2")
            nc.sync.dma_start(out=W2T_t[:], in_=W2T_d[:])
            b1_t = cpool.tile([P, H_FEATS], f32, tag="b1")
            nc.sync.dma_start(out=b1_t[:], in_=b1_d[:])
            b2_t = cpool.tile([P, H_FEATS], f32, tag="b2")
            nc.sync.dma_start(out=b2_t[:], in_=b2_d[:])
            ident_t = cpool.tile([P, P], f32, tag="ident")
            nc.sync.dma_start(out=ident_t[:], in_=ident_d[:])

            y1_own = y2_own = y1_bounce = y2_bounce = y1_full = y2_full = None

            # A chunk covers ranges [0, SPLIT_R) of every core, B the rest;
            # each is a contiguous AllGather output issued as soon as its
            # producing ranges finish, overlapping the remaining compute.
            AG_SPLITS = ([(0, SPLIT_R), (SPLIT_R, NRANGES)] if AG_CHUNKED
                         else [(0, NRANGES)])

            def ag_chunk(bounce, full_ab, r0, r1):
                a, b = r0 * P, min(r1 * P, NB)
                full = full_ab[0] if r0 == 0 else full_ab[1]
                if no_collectives:
                    nc.sync.dma_start(out=full[0:b - a, :], in_=bounce[a:b, :])
                else:
                    nc.gpsimd.collective_compute(
                        "AllGather", mybir.AluOpType.bypass,
                        replica_groups=[list(range(NCORES))],
                        ins=[bounce[a:b, :].opt()], outs=[full[:].opt()])

            def one_rep():
                # dram intermediates rotate (bufs=2) so rep k+1's phase A
                # can overlap rep k's tail instead of serializing on WAR
                nonlocal y1_own, y2_own, y1_bounce, y2_bounce, y1_full, y2_full
                y1_own = dram.tile([NB, H_FEATS], f32, tag="y1own")
                y2_own = dram.tile([NB, H_FEATS], f32, tag="y2own")
                y1_bounce = dram.tile([NB, H_FEATS], mdt, tag="y1b")
                y2_bounce = dram.tile([NB, H_FEATS], mdt, tag="y2b")
                if AG_CHUNKED:
                    y1_full = (dram.tile([ROWS_A, H_FEATS], mdt, tag="y1fA",
                                         name="y1fA"),
                               dram.tile([ROWS_B, H_FEATS], mdt, tag="y1fB",
                                         name="y1fB"))
                    y2_full = (dram.tile([ROWS_A, H_FEATS], mdt, tag="y2fA",
                                         name="y2fA"),
                               dram.tile([ROWS_B, H_FEATS], mdt, tag="y2fB",
                                         name="y2fB"))
                else:
                    # Shared addr space: the recommended fast path for
                    # HBM-HBM AllGather outputs
                    y1_full = (dram.tile([N, H_FEATS], mdt, tag="y1f",
                                         name="y1f", addr_space="Shared"),)
                    y2_full = (dram.tile([N, H_FEATS], mdt, tag="y2f",
                                         name="y2f", addr_space="Shared"),)
                # ------------ phase A: y1 = hT.T @ W1T (own block)
                phase_a()
                layer(y1_full, y1_own, b1_t, l1_out, ag_out=(y2_bounce, y2_full))
                layer(y2_full, y2_own, b2_t, l2_out)

            def phase_a():
                splits = dict((r1 - 1, (r0, r1)) for r0, r1 in AG_SPLITS)
                r = 0
                while r < NRANGES:
                    if (r + 1 < NRANGES and (r + 2) * P <= NB
                            and r not in splits and r + 1 not in splits):
                        phase_a_pair(r)
                        r += 2
                        continue
                    phase_a_range(r)
                    if r in splits:
                        ag_chunk(y1_bounce, y1_full, *splits[r])
                    r += 1

            def phase_a_pair(r):
                # two full ranges per iteration: halves the DMA/DVE
                # instruction count of phase A
                hT_t = [sb.tile([P, 2 * P], f32, tag=f"hTp{ch}",
                                name=f"hTp{ch}")
                        for ch in range(IN_FEATS // P)]
                for ch in range(IN_FEATS // P):
                    nc.sync.dma_start(
                        out=hT_t[ch][:],
                        in_=hT_d[ch * P:(ch + 1) * P, r * P:(r + 2) * P])
                y1_ps = ps_a.tile([P, 2, H_FEATS], f32, tag="y1psp")
                for sub in range(2):
                    for ch in range(IN_FEATS // P):
                        nc.tensor.matmul(
                            out=y1_ps[:, sub, :],
                            lhsT=hT_t[ch][:, sub * P:(sub + 1) * P],
                            rhs=W1T_t[ch][:],
                            start=(ch == 0), stop=(ch == IN_FEATS // P - 1))
                y1_sb = sb.tile([P, 2, H_FEATS], f32, tag="y1sbp")
                nc.vector.tensor_tensor(
                    out=y1_sb[:], in0=y1_ps[:],
                    in1=b1_t[:].unsqueeze(1).broadcast_to([P, 2, H_FEATS]),
                    op=mybir.AluOpType.add)
                nc.sync.dma_start(
                    out=y1_own[r * P:(r + 2) * P, :
                               ].rearrange("(s p) h -> p s h", p=P),
                    in_=y1_sb[:])
                y1_sbh = sb.tile([P, 2, H_FEATS], mdt, tag="y1sbhp")
                nc.vector.tensor_copy(out=y1_sbh[:], in_=y1_ps[:])
                nc.sync.dma_start(
                    out=y1_bounce[r * P:(r + 2) * P, :
                                  ].rearrange("(s p) h -> p s h", p=P),
                    in_=y1_sbh[:])

            def phase_a_range(r):
                    rows = min(P, NB - r * P)
                    hT_t = [sb.tile([P, P], f32, tag=f"hT{ch}", name=f"hT{ch}")
                            for ch in range(IN_FEATS // P)]
                    for ch in range(IN_FEATS // P):
                        nc.sync.dma_start(
                            out=hT_t[ch][:, :rows],
                            in_=hT_d[ch * P:(ch + 1) * P, r * P:r * P + rows])
                    y1_psp = ps_a.tile([P, 2, H_FEATS], f32, tag="y1psp")
                    y1_ps = y1_psp[:, 0, :]
                    for ch in range(IN_FEATS // P):
                        nc.tensor.matmul(
                            out=y1_ps, lhsT=hT_t[ch][:], rhs=W1T_t[ch][:],
                            start=(ch == 0), stop=(ch == IN_FEATS // P - 1))
                    # y1_own stores y1 + b1 (bias folded in; the bounce/gather
                    # table keeps plain y1)
                    y1_sb = sb.tile([P, H_FEATS], f32, tag="y1sb")
                    nc.vector.tensor_tensor(out=y1_sb[:], in0=y1_ps,
                                            in1=b1_t[:], op=mybir.AluOpType.add)
                    nc.sync.dma_start(out=y1_own[r * P:r * P + rows, :],
                                      in_=y1_sb[:rows, :])
                    if MSG_BF16:
                        y1_sbh = sb.tile([P, H_FEATS], mdt, tag="y1sbh")
                        nc.vector.tensor_copy(out=y1_sbh[:], in_=y1_ps)
                        nc.sync.dma_start(out=y1_bounce[r * P:r * P + rows, :],
                                          in_=y1_sbh[:rows, :])
                    else:
                        nc.sync.dma_start(out=y1_bounce[r * P:r * P + rows, :],
                                          in_=y1_sb[:rows, :])

            def layer(y_full_t, y_own_t, b_t, h_out_cb, ag_out=None):
                splits = dict((r1 - 1, (r0, r1)) for r0, r1 in AG_SPLITS)
                goff = [0] * NGROUPS
                for batch in batches:
                    slabs = []
                    for g in range(NGROUPS):
                        ntiles = int(sum(T[r, g] for r in batch))
                        nidx = ntiles * P
                        idx_t = sb.tile([P, nidx // 16], i16, tag=f"idx{g}")
                        nc.sync.dma_start(
                            out=idx_t[:],
                            in_=idx_d[g][:, goff[g] * 8:(goff[g] + ntiles) * 8])
                        slab = slabp.tile([P, ntiles, H_FEATS], mdt,
                                          tag=f"slab{g}")
                        if "gather" in skip:
                            nc.vector.memset(slab[:, 0:1, :], 0)
                        else:
                            src_t = (y_full_t[0] if (not AG_CHUNKED or g < 2)
                                     else y_full_t[1])
                            nc.gpsimd.dma_gather(
                                out_ap=slab[:],
                                in_ap=src_t[GBASE[g]:GBASE[g] + GSIZE[g], :],
                                idxs_ap=idx_t[:], num_idxs=nidx, num_idxs_reg=nidx,
                                elem_size=H_FEATS, single_packet=False,
                                queue_num=g)
                        slabs.append(slab)

                    tilebase = [0] * NGROUPS
                    for r in batch:
                        ct = int(CT[r])
                        rows = min(P, NB - r * P)
                        S = sb.tile([P, ct, P], mybir.dt.float8e4, tag="S")
                        nc.sync.dma_start(
                            out=S[:],
                            in_=S_d[:, int(CTOFF[r]) * P:(int(CTOFF[r]) + ct) * P
                                    ].rearrange("p (t s) -> p t s", s=P))

                        agg_ps = ps_agg.tile([P, H_FEATS], f32, tag="aggps")
                        if "matmul" in skip:
                            nc.tensor.matmul(
                                out=agg_ps[:], lhsT=S[:, 0, :],
                                rhs=S[:, 0, :],
                                start=True, stop=True)
                            for g in range(NGROUPS):
                                tilebase[g] += int(T[r, g])
                        else:
                            ncells = 0
                            for g in range(NGROUPS):
                                tg = int(T[r, g])
                                colbase = int(T[r, :g].sum())
                                for j in range(tg):
                                    nc.tensor.matmul(
                                        out=agg_ps[:],
                                        lhsT=S[:, colbase + j, :],
                                        rhs=slabs[g][:, tilebase[g] + j, :],
                                        start=(ncells == 0),
                                        stop=(ncells == int(CT[r]) - 1))
                                    ncells += 1
                                tilebase[g] += tg

                        yown_t = sb.tile([P, H_FEATS], f32, tag="yown")
                        nc.sync.dma_start(out=yown_t[:rows, :],
                                          in_=y_own_t[r * P:r * P + rows, :])
                        z = sb.tile([P, H_FEATS], f32, tag="z")
                        nc.vector.scalar_tensor_tensor(
                            out=z[:], in0=agg_ps[:],
                            scalar=invdegT_t[:, r:r + 1], in1=yown_t[:],
                            op0=mybir.AluOpType.mult, op1=mybir.AluOpType.add)
                        h_t = sb.tile([P, H_FEATS], f32, tag="h")
                        nc.scalar.activation(
                            out=h_t[:], in_=z[:],
                            func=mybir.ActivationFunctionType.Relu)
                        h_out_cb(r, rows, h_t)
                        if ag_out is not None and r in splits:
                            ag_chunk(*ag_out, *splits[r])
                    for g in range(NGROUPS):
                        goff[g] += int(sum(T[r, g] for r in batch))

            def l1_out(r, rows, h_t):
                h1T_ps = ps_tr.tile([P, P], f32, tag="h1Tps")
                nc.tensor.transpose(out=h1T_ps[:], in_=h_t[:],
                                    identity=ident_t[:])
                h1T_sb = sb.tile([P, P], f32, tag="h1Tsb")
                nc.scalar.activation(out=h1T_sb[:], in_=h1T_ps[:],
                                     func=mybir.ActivationFunctionType.Copy)
                y2_ps = ps_y2.tile([P, H_FEATS], f32, tag="y2ps")
                nc.tensor.matmul(out=y2_ps[:], lhsT=h1T_sb[:], rhs=W2T_t[:],
                                 start=True, stop=True)
                # y2_own stores y2 + b2 (bias folded; bounce keeps plain y2)
                y2_sb = sb.tile([P, H_FEATS], f32, tag="y2sb")
                nc.vector.tensor_tensor(out=y2_sb[:], in0=y2_ps[:],
                                        in1=b2_t[:], op=mybir.AluOpType.add)
                nc.sync.dma_start(out=y2_own[r * P:r * P + rows, :],
                                  in_=y2_sb[:rows, :])
                if MSG_BF16:
                    y2_sbh = sb.tile([P, H_FEATS], mdt, tag="y2sbh")
                    nc.vector.tensor_copy(out=y2_sbh[:], in_=y2_ps[:])
                    nc.sync.dma_start(out=y2_bounce[r * P:r * P + rows, :],
                                      in_=y2_sbh[:rows, :])
                else:
                    nc.sync.dma_start(out=y2_bounce[r * P:r * P + rows, :],
                                      in_=y2_sb[:rows, :])

            def l2_out(r, rows, h_t):
                nc.sync.dma_start(out=out_d[r * P:r * P + rows, :],
                                  in_=h_t[:rows, :])

            for _ in range(reps):
                one_rep()

    nc.compile()
    return nc


# ----------------------------------------------------------------- runner

def make_runner(nc, in_maps):
    """Reusable sharded executable over 8 cores (mirrors
    bass2jax.run_bass_via_pjrt but keeps the jitted fn + staged inputs).
    Returns (run, time_once) where run() -> list[dict] of outputs and
    time_once() -> wall seconds for one steady-state execution."""
    import jax
    from jax.sharding import Mesh, PartitionSpec
    from jax.experimental.shard_map import shard_map
    import concourse.mybir as mb

    bass2jax.install_neuronx_cc_hook()
    n_cores = len(in_maps)

    partition_name = (nc.partition_id_tensor.name
                      if nc.partition_id_tensor else None)
    in_names, out_names, out_avals, zero_outs = [], [], [], []
    for alloc in nc.m.functions[0].allocations:
        if not isinstance(alloc, mb.MemoryLocationSet):
            continue
        name = alloc.memorylocations[0].name
        if alloc.kind == "ExternalInput":
            if name != partition_name:
                in_names.append(name)
        elif alloc.kind == "ExternalOutput":
            out_names.append(name)
            shape = tuple(alloc.tensor_shape)
            dtype = mb.dt.np(alloc.dtype)
            out_avals.append(jax.core.ShapedArray(shape, dtype))
            zero_outs.append(np.zeros(shape, dtype))
    n_params = len(in_names)
    n_outs = len(out_avals)
    in_names_full = list(in_names) + out_names
    if partition_name is not None:
        in_names_full.append(partition_name)

    def _body(*args):
        operands = list(args)
        if partition_name is not None:
            operands.append(bass2jax.partition_id_tensor())
        outs = bass2jax._bass_exec_p.bind(
            *operands,
            out_avals=tuple(out_avals),
            in_names=tuple(in_names_full),
            out_names=tuple(out_names),
            lowering_input_output_aliases=(),
            sim_require_finite=True,
            sim_require_nnan=True,
            nc=nc,
        )
        return tuple(outs)

    devices = jax.devices()[:n_cores]
    mesh = Mesh(np.asarray(devices), ("core",))
    in_specs = (PartitionSpec("core"),) * (n_params + n_outs)
    out_specs = (PartitionSpec("core"),) * n_outs
    donate = tuple(range(n_params, n_params + n_outs))
    sharded = jax.jit(
        shard_map(_body, mesh=mesh, in_specs=in_specs, out_specs=out_specs,
                  check_rep=False),
        donate_argnums=donate, keep_unused=True)

    concat_in = [
        np.concatenate([np.asarray(in_maps[c][nm]) for c in range(n_cores)], 0)
        for nm in in_names]
    sharding = jax.sharding.NamedSharding(mesh, PartitionSpec("core"))
    staged = [jax.device_put(a, sharding) for a in concat_in]

    def _zeros():
        return [jax.device_put(
            np.zeros((n_cores * z.shape[0], *z.shape[1:]), z.dtype), sharding)
            for z in zero_outs]

    def run():
        out_arrs = sharded(*staged, *_zeros())
        jax.block_until_ready(out_arrs)
        return [
            {nm: np.asarray(out_arrs[i]).reshape(n_cores, *out_avals[i].shape)[c]
             for i, nm in enumerate(out_names)}
            for c in range(n_cores)]

    def time_once():
        zs = _zeros()
        jax.block_until_ready(zs)
        t0 = time.perf_counter()
        out_arrs = sharded(*staged, *zs)
        jax.block_until_ready(out_arrs)
        return time.perf_counter() - t0

    def time_slope(k=16):
        """Marginal device time per execution: queue k+1 executions without
        intermediate sync; slope vs a single execution."""
        zsets = [_zeros() for _ in range(k + 1)]
        for zs in zsets:
            jax.block_until_ready(zs)
        outs = sharded(*staged, *zsets[0])
        jax.block_until_ready(outs)          # warm
        t0 = time.perf_counter()
        outs = sharded(*staged, *zsets[1])
        jax.block_until_ready(outs)
        t1 = time.perf_counter()
        last = None
        for i in range(2, k + 1):
            last = sharded(*staged, *zsets[i])
        jax.block_until_ready(last)
        t2 = time.perf_counter()
        single = t1 - t0
        per = (t2 - t1) / (k - 1)
        return single, per

    return run, time_once, time_slope


def kernel(h, W1, b1, W2, b2, src, dst):
    h = np.asarray(h, np.float32)
    in_maps, meta = preprocess(h, np.asarray(W1, np.float32),
                               np.asarray(b1, np.float32),
                               np.asarray(W2, np.float32),
                               np.asarray(b2, np.float32), src, dst)
    nc = build_program(meta)
    run, _, _ = make_runner(nc, in_maps)
    results = run()
    out_new = np.concatenate([results[c]["out"] for c in range(NCORES)], 0)
    return out_new[meta["perm"]].astype(np.float32)

